# revision 12
# baseline (speedup 1.0000x reference)
"""Trainium2 Bass kernel for CSHA attention (ECA channel + spatial attention).

out = x * (1 + ch_w[c] + sp[h, w]) for x [B, C, H, W] = [32, 256, 64, 64].

Per core (4 batches):
  ACT : cast x f32->f16 (+ accum_out channel sums for ECA), S' PSUM->SBUF
        chunk copies with bias = 1 + chw_g[c] fused in
  DVE : g-folds (max / in-place add), TRANSPOSE_TENSOR_REDUCE (32-group
        partition reduction), small f16 map-building ops, 1 of 4 final
        multiply chunks (stt from PSUM)
  PE  : 7x7 conv (10 banded matmuls), ECA conv1d, sp transpose, S'
        broadcast via ones-matmul (f16)
  Pool: 3 of 4 final multiply chunks (tensor_mul, in place over x)
  DMA : all 8 batch-half loads dispatched first; per-chunk stores
"""

import sys

import numpy as np

sys.path.insert(0, "/opt/trn_rl_repo")

B, C, H, W = 32, 256, 64, 64
HW = H * W            # 4096
N_CORES = 8
BPC = B // N_CORES    # 4 batches per core


def _build_host_consts(conv1d_w, conv2d_w, conv2d_b):
    w5 = np.asarray(conv1d_w, np.float32)[0, 0]           # [5]
    W1 = np.zeros((C, C), np.float32)
    for k in range(5):
        co = np.arange(C)
        ci = co + k - 2
        m = (ci >= 0) & (ci < C)
        W1[co[m], ci[m]] = w5[k] / HW
    w1t = W1.T.reshape(2, 128, C).transpose(1, 0, 2).astype(np.float16)

    wt = np.asarray(conv2d_w, np.float32)[0].copy()       # [2, 7, 7]
    wt[1] /= C
    wd = np.zeros((128, 10, 128), np.float32)
    wi = np.arange(64)
    for ch in range(2):
        for di, d in enumerate(range(-2, 3)):
            M = np.zeros((128, 128), np.float32)
            for hp in range(2):
                for hpp in range(2):
                    dy = 2 * d + hp - hpp + 3
                    if not (0 <= dy <= 6):
                        continue
                    for dx in range(7):
                        w_out = wi
                        w_in = w_out + dx - 3
                        msk = (w_in >= 0) & (w_in < 64)
                        M[hp * 64 + w_in[msk], hpp * 64 + w_out[msk]] = wt[ch, dy, dx]
            wd[:, ch * 5 + di, :] = M
    bias = np.full((128, 1), float(np.asarray(conv2d_b)[0]), np.float32)
    return w1t, wd.astype(np.float16), bias


def _build_nc():
    import concourse.bass as bass
    import concourse.tile as tile
    from concourse import mybir

    f32 = mybir.dt.float32
    f16 = mybir.dt.float16

    nc = bass.Bass()

    xs_d = nc.dram_tensor("xs", [BPC, C, H, W], f32, kind="ExternalInput")
    w1t_d = nc.dram_tensor("w1t", [128, 2, C], f16, kind="ExternalInput")
    wd_d = nc.dram_tensor("wd", [128, 10, 128], f16, kind="ExternalInput")
    bias_d = nc.dram_tensor("bias", [128, 1], f32, kind="ExternalInput")
    out_d = nc.dram_tensor("out", [BPC, C, H, W], f32, kind="ExternalOutput")

    i128b_d = nc.inline_tensor(np.eye(128, dtype=np.float16), "i128b")
    ones1_d = nc.inline_tensor(np.ones((1, 128), np.float16), "ones1")

    AX = mybir.AxisListType
    ALU = mybir.AluOpType
    ACT = mybir.ActivationFunctionType

    with tile.TileContext(nc) as tc:
        with (
            tc.tile_pool(name="consts", bufs=1) as consts,
            tc.tile_pool(name="xp", bufs=4) as xp,
            tc.tile_pool(name="scrp", bufs=2) as scrp,
            tc.tile_pool(name="m1p", bufs=1) as m1p,
            tc.tile_pool(name="work", bufs=2) as work,
            tc.tile_pool(name="mapp", bufs=1) as mapp,
            tc.tile_pool(name="rrp", bufs=2) as rrp,
            tc.tile_pool(name="ssb", bufs=2) as ssb,
            tc.tile_pool(name="psb", bufs=3, space=bass.MemorySpace.PSUM) as psb,
            tc.tile_pool(name="pss", bufs=2, space=bass.MemorySpace.PSUM) as pss,
        ):
            # ---- constants first (tiny; must not queue behind the bulk
            # x loads — PE conv/bcast and ACT sigmoid all gate on them) ----
            w1t_t = consts.tile([128, 2, C], f16)
            nc.sync.dma_start(out=w1t_t, in_=w1t_d[:])
            wd_t = consts.tile([128, 10, 128], f16)
            nc.sync.dma_start(out=wd_t, in_=wd_d[:])
            bias_t = consts.tile([128, 1], f32)
            nc.sync.dma_start(out=bias_t, in_=bias_d[:])
            i128b_t = consts.tile([128, 128], f16)
            nc.sync.dma_start(out=i128b_t, in_=i128b_d[:])
            ones1_t = consts.tile([1, 128], f16)
            nc.sync.dma_start(out=ones1_t, in_=ones1_d[:])

            # ---- all batch loads dispatched up front (per g half) ----
            x_tiles = []
            for b in range(BPC):
                x_t = xp.tile([128, 2, HW], f32, tag="x")
                x_tiles.append(x_t)
                for u in range(2):
                    for g in range(2):
                        nc.sync.dma_start(
                            out=x_t[:, g, u * 2048 : (u + 1) * 2048],
                            in_=xs_d[b, 128 * g : 128 * (g + 1)].rearrange(
                                "c h w -> c (h w)"
                            )[:, u * 2048 : (u + 1) * 2048],
                        )

            # Dummy matmuls absorb const-load DMA waits on PE early.
            pd = pss.tile([1, 4], f32, tag="sm")
            for k, (lhs, rhs) in enumerate((
                (w1t_t[:, 0, 0:1], w1t_t[:, 0, 1:2]),
                (wd_t[:, 0, 0:1], wd_t[:, 0, 1:2]),
                (i128b_t[:, 0:1], i128b_t[:, 1:2]),
            )):
                nc.tensor.matmul(pd[:, k : k + 1], lhs, rhs, start=True, stop=True)
            junk0 = work.tile([128, 1], f32, tag="junk")
            nc.scalar.activation(out=junk0, in_=bias_t, func=ACT.Copy)

            # Interleaved conv maps: pads (cols 0:2, 34:36) zeroed once;
            # data cols rewritten every batch.
            mp_map = mapp.tile([128, 36], f16, tag="mp")
            ap_map = mapp.tile([128, 36], f16, tag="ap")
            for mp in (mp_map, ap_map):
                nc.vector.memset(
                    mp.rearrange("p (u c) -> p u c", u=18)[:, 0:18:17, :], 0.0
                )

            for b in range(BPC):
                x_t = x_tiles[b]

                # ---- ACT cast f32->f16 + ECA channel sums over hw ----
                # Split into 2048-wide pieces: long ACT ops otherwise block
                # the (high-priority) tail ops of the previous batch.
                scr = scrp.tile([128, 2, HW], f16, tag="scr")
                yb8 = work.tile([128, 2, 2], f32, tag="yb8")
                for u in range(2):
                    for g in range(2):
                        sl = slice(u * 2048, (u + 1) * 2048)
                        nc.scalar.activation(
                            out=scr[:, g, sl],
                            in_=x_t[:, g, sl],
                            func=ACT.Copy,
                            accum_out=yb8[:, g, u : u + 1],
                        )
                yb = work.tile([128, 2], f32, tag="yb")
                nc.vector.tensor_add(yb, yb8[:, :, 0], yb8[:, :, 1])

                # ---- g-fold + transpose-reduce, max path per half ----
                # R[32a+i, v] = red_j in[32a+j, 32v+i]   (pixel hw = 32v+i)
                # Half-size fold buffer rotates: fold(u) -> TR(u).
                Rm = mapp.tile([128, 128], f16, tag="Rm")
                Ra = mapp.tile([128, 128], f32, tag="Ra")
                for u in range(2):
                    sl = slice(u * 2048, (u + 1) * 2048)
                    fh = m1p.tile([128, 2048], f16, tag="m1")
                    nc.vector.tensor_max(fh, scr[:, 0, sl], scr[:, 1, sl])
                    nc.vector.tensor_reduce(
                        out=Rm[:, u * 64 : (u + 1) * 64],
                        in_=fh.rearrange("p (v j) -> p v j", j=32),
                        axis=AX.X, op=ALU.max, apply_transpose=True,
                    )
                for u in range(2):
                    sl = slice(u * 2048, (u + 1) * 2048)
                    nc.vector.tensor_add(scr[:, 0, sl], scr[:, 0, sl], scr[:, 1, sl])
                for u in range(2):
                    nc.vector.tensor_reduce(
                        out=Ra[:, u * 64 : (u + 1) * 64],
                        in_=scr[:, 0, u * 2048 : (u + 1) * 2048].rearrange(
                            "p (v j) -> p v j", j=32
                        ),
                        axis=AX.X, op=ALU.add, apply_transpose=True,
                    )

                # ---- finish reduction over the 4 a-groups; build maps ----
                # map[32q+i, 2+h2] = red_a R[32a+i, 4h2+q]
                # The whole per-batch tail (small map ops -> conv -> S' ->
                # multiply -> store) runs at high priority so the scheduler
                # drains batch b's latency chain before batch b+1's bulk
                # fold/reduce work.
                tail_ctx = tc.high_priority()
                tail_ctx.__enter__()
                for path, (R, top) in enumerate(((Rm, ALU.max), (Ra, ALU.add))):
                    G = mapp.tile([32, 4, 128], f16, tag="G")
                    for a in range(4):
                        nc.scalar.activation(
                            out=G[:, a, :], in_=R[32 * a : 32 * a + 32, :],
                            func=ACT.Copy,
                        )
                    G2 = mapp.tile([32, 2, 128], f16, tag="G2")
                    nc.vector.tensor_tensor(
                        out=G2.rearrange("p t v -> p (t v)"),
                        in0=G[:, 0:2, :].rearrange("p t v -> p (t v)"),
                        in1=G[:, 2:4, :].rearrange("p t v -> p (t v)"),
                        op=top,
                    )
                    # Final fold writes map stripes directly (output at
                    # partition base 32q): map[32q+i, 2+h2] = red over t of
                    # G2[i, t, 4h2+q].
                    mp = mp_map if path == 0 else ap_map
                    for q in range(4):
                        nc.vector.tensor_tensor(
                            out=mp[32 * q : 32 * q + 32, 2:34],
                            in0=G2[:, 0, q : 125 + q : 4],
                            in1=G2[:, 1, q : 125 + q : 4],
                            op=top,
                        )

                # ---- spatial conv (10 accumulated matmuls) + sigmoid ----
                psp = pss.tile([128, 32], f32, tag="sm")
                for ch in range(2):
                    mm = mp_map if ch == 0 else ap_map
                    for di in range(5):
                        nc.tensor.matmul(
                            psp,
                            wd_t[:, ch * 5 + di, :],
                            mm[:, di : di + 32],
                            start=(ch == 0 and di == 0),
                            stop=(ch == 1 and di == 4),
                        )
                sp16 = work.tile([128, 32], f16, tag="sp16")
                nc.scalar.activation(
                    out=sp16, in_=psp, func=ACT.Sigmoid, bias=bias_t[:, 0:1]
                )

                # ---- sp row: transpose + flatten DMA -> rrow [1, HW] ----
                pspT = pss.tile([32, 128], f16, tag="sm")
                nc.tensor.transpose(out=pspT, in_=sp16, identity=i128b_t)
                fr = work.tile([32, 128], f16, tag="fr")
                nc.scalar.activation(out=fr, in_=pspT, func=ACT.Identity, bias=1.0)
                rrow = rrp.tile([1, HW], f16, tag="rrow")
                nc.sync.dma_start(out=rrow, in_=fr)

                # ---- ECA conv1d + sigmoid -> chw1 = 1 + sigmoid(...) ----
                ybh = work.tile([128, 2], f16, tag="ybh")
                nc.scalar.activation(out=ybh, in_=yb, func=ACT.Copy)
                pchw = pss.tile([128, 2], f32, tag="sm")
                for hp in range(2):
                    for kh in range(2):
                        nc.tensor.matmul(
                            pchw[:, hp : hp + 1],
                            w1t_t[:, kh, hp * 128 : (hp + 1) * 128],
                            ybh[:, kh : kh + 1],
                            start=(kh == 0),
                            stop=(kh == 1),
                        )
                chw1 = work.tile([128, 2], f32, tag="chw")
                nc.scalar.activation(out=chw1, in_=pchw, func=ACT.Sigmoid)

                # ---- S' broadcast (PE, per 1024) + DVE stt multiply ----
                # stt: out = (S_psum + chw1_g) * x, in place over x.  All
                # multiplies on DVE (the dominant stream); no SBUF S' copies.
                tail_ctx.__exit__(None, None, None)
                for g in range(2):
                    for cc in range(2):
                        if cc == 1 and b < BPC - 1:
                            # Pool chunk: PSUM -> SBUF via ACT (bias adds
                            # 1 + chw), gpsimd multiply, store
                            xsl = x_t[:, g, cc * 2048 : (cc + 1) * 2048]
                            Sb = ssb.tile([128, 2048], f32, tag="Sb")
                            for half in range(2):
                                ps = psb.tile([128, 1024], f32, tag="bc")
                                for k in range(2):
                                    col = cc * 2048 + half * 1024 + k * 512
                                    nc.tensor.matmul(
                                        ps[:, k * 512 : (k + 1) * 512],
                                        ones1_t,
                                        rrow[:, col : col + 512],
                                        start=True, stop=True,
                                    )
                                nc.scalar.activation(
                                    out=Sb[:, half * 1024 : (half + 1) * 1024],
                                    in_=ps,
                                    func=ACT.Identity,
                                    bias=chw1[:, g : g + 1],
                                )
                            nc.gpsimd.tensor_mul(xsl, xsl, Sb)
                            nc.sync.dma_start(
                                out=out_d[b, 128 * g : 128 * (g + 1)]
                                .rearrange("c h w -> c (h w)")
                                [:, cc * 2048 : (cc + 1) * 2048],
                                in_=xsl,
                            )
                            continue
                        for half in range(2):
                            ps = psb.tile([128, 1024], f32, tag="bc")
                            for k in range(2):
                                col = cc * 2048 + half * 1024 + k * 512
                                nc.tensor.matmul(
                                    ps[:, k * 512 : (k + 1) * 512],
                                    ones1_t,
                                    rrow[:, col : col + 512],
                                    start=True, stop=True,
                                )
                            xh = x_t[:, g, cc * 2048 + half * 1024 :
                                     cc * 2048 + (half + 1) * 1024]
                            nc.vector.scalar_tensor_tensor(
                                out=xh,
                                in0=ps,
                                scalar=chw1[:, g : g + 1],
                                in1=xh,
                                op0=ALU.add,
                                op1=ALU.mult,
                            )
                        nc.sync.dma_start(
                            out=out_d[b, 128 * g : 128 * (g + 1)]
                            .rearrange("c h w -> c (h w)")
                            [:, cc * 2048 : (cc + 1) * 2048],
                            in_=x_t[:, g, cc * 2048 : (cc + 1) * 2048],
                        )

    _split_excess_waits(nc, mybir)
    return nc


def _split_excess_waits(nc, mybir):
    """Walrus limits sync-wait commands per instruction.  Move excess waits
    onto an inserted same-engine NoOp immediately before the instruction."""
    SKIP = (mybir.InstNoOp, mybir.InstAllEngineBarrier)
    for fn in nc.m.functions:
        for blk in fn.blocks:
            new = []
            for inst in blk.instructions:
                si = inst.sync_info
                if si is not None and si.on_wait and not isinstance(inst, SKIP):
                    waits = list(si.on_wait)
                    if len(waits) > 1:
                        moved, keep = waits[:-1], waits[-1:]
                        for k, w in enumerate(moved):
                            nop = mybir.InstNoOp(
                                name=f"{inst.name}-wsplit{k}",
                                engine=inst.engine,
                                sync_info=mybir.SyncInfo(on_wait=[w], on_update=[]),
                                bass_nofuse=True,
                            )
                            new.append(nop)
                        si.on_wait = keep
                new.append(inst)
            blk.instructions[:] = new


def kernel(x, conv1d_w, conv2d_w, conv2d_b):
    x = np.ascontiguousarray(np.asarray(x, np.float32))
    w1t, wd, bias = _build_host_consts(conv1d_w, conv2d_w, conv2d_b)

    from concourse.bass_utils import run_bass_kernel_spmd

    nc = _build_nc()
    shards = x.reshape(N_CORES, BPC, C, H, W)
    in_maps = [
        {"xs": np.ascontiguousarray(shards[i]), "w1t": w1t, "wd": wd, "bias": bias}
        for i in range(N_CORES)
    ]
    res = run_bass_kernel_spmd(nc, in_maps, core_ids=list(range(N_CORES)))
    out = np.concatenate([r["out"] for r in res.results], axis=0)
    return out.reshape(B, C, H, W)


# revision 14
# speedup vs baseline: 1.0735x; 1.0735x over previous
"""Trainium2 Bass kernel for CSHA attention (ECA channel + spatial attention).

out = x * (1 + ch_w[c] + sp[h, w]) for x [B, C, H, W] = [32, 256, 64, 64].

Per core (4 batches):
  ACT : cast x f32->f16 (+ accum_out channel sums for ECA), S' PSUM->SBUF
        chunk copies with bias = 1 + chw_g[c] fused in
  DVE : g-folds (max / in-place add), TRANSPOSE_TENSOR_REDUCE (32-group
        partition reduction), small f16 map-building ops, 1 of 4 final
        multiply chunks (stt from PSUM)
  PE  : 7x7 conv (10 banded matmuls), ECA conv1d, sp transpose, S'
        broadcast via ones-matmul (f16)
  Pool: 3 of 4 final multiply chunks (tensor_mul, in place over x)
  DMA : all 8 batch-half loads dispatched first; per-chunk stores
"""

import sys

import numpy as np

sys.path.insert(0, "/opt/trn_rl_repo")

B, C, H, W = 32, 256, 64, 64
HW = H * W            # 4096
N_CORES = 8
BPC = B // N_CORES    # 4 batches per core


def _build_host_consts(conv1d_w, conv2d_w, conv2d_b):
    w5 = np.asarray(conv1d_w, np.float32)[0, 0]           # [5]
    W1 = np.zeros((C, C), np.float32)
    for k in range(5):
        co = np.arange(C)
        ci = co + k - 2
        m = (ci >= 0) & (ci < C)
        W1[co[m], ci[m]] = w5[k] / HW
    w1t = W1.T.reshape(2, 128, C).transpose(1, 0, 2).astype(np.float16)

    wt = np.asarray(conv2d_w, np.float32)[0].copy()       # [2, 7, 7]
    wt[1] /= C
    wd = np.zeros((128, 10, 128), np.float32)
    wi = np.arange(64)
    for ch in range(2):
        for di, d in enumerate(range(-2, 3)):
            M = np.zeros((128, 128), np.float32)
            for hp in range(2):
                for hpp in range(2):
                    dy = 2 * d + hp - hpp + 3
                    if not (0 <= dy <= 6):
                        continue
                    for dx in range(7):
                        w_out = wi
                        w_in = w_out + dx - 3
                        msk = (w_in >= 0) & (w_in < 64)
                        M[hp * 64 + w_in[msk], hpp * 64 + w_out[msk]] = wt[ch, dy, dx]
            wd[:, ch * 5 + di, :] = M
    bias = np.full((128, 1), float(np.asarray(conv2d_b)[0]), np.float32)
    return w1t, wd.astype(np.float16), bias


def _build_nc():
    import concourse.bass as bass
    import concourse.tile as tile
    from concourse import mybir

    f32 = mybir.dt.float32
    f16 = mybir.dt.float16

    nc = bass.Bass()

    xs_d = nc.dram_tensor("xs", [BPC, C, H, W], f32, kind="ExternalInput")
    w1t_d = nc.dram_tensor("w1t", [128, 2, C], f16, kind="ExternalInput")
    wd_d = nc.dram_tensor("wd", [128, 10, 128], f16, kind="ExternalInput")
    bias_d = nc.dram_tensor("bias", [128, 1], f32, kind="ExternalInput")
    out_d = nc.dram_tensor("out", [BPC, C, H, W], f32, kind="ExternalOutput")

    i128b_d = nc.inline_tensor(np.eye(128, dtype=np.float16), "i128b")
    ones1_d = nc.inline_tensor(np.ones((1, 128), np.float16), "ones1")

    AX = mybir.AxisListType
    ALU = mybir.AluOpType
    ACT = mybir.ActivationFunctionType

    with tile.TileContext(nc) as tc:
        with (
            tc.tile_pool(name="consts", bufs=1) as consts,
            tc.tile_pool(name="xp", bufs=4) as xp,
            tc.tile_pool(name="scrp", bufs=2) as scrp,
            tc.tile_pool(name="m1p", bufs=1) as m1p,
            tc.tile_pool(name="work", bufs=2) as work,
            tc.tile_pool(name="mapp", bufs=2) as mapp,
            tc.tile_pool(name="rrp", bufs=2) as rrp,
            tc.tile_pool(name="ssb", bufs=1) as ssb,
            tc.tile_pool(name="psb", bufs=3, space=bass.MemorySpace.PSUM) as psb,
            tc.tile_pool(name="pss", bufs=2, space=bass.MemorySpace.PSUM) as pss,
        ):
            # ---- constants first (tiny; must not queue behind the bulk
            # x loads — PE conv/bcast and ACT sigmoid all gate on them) ----
            w1t_t = consts.tile([128, 2, C], f16)
            nc.sync.dma_start(out=w1t_t, in_=w1t_d[:])
            wd_t = consts.tile([128, 10, 128], f16)
            nc.sync.dma_start(out=wd_t, in_=wd_d[:])
            bias_t = consts.tile([128, 1], f32)
            nc.sync.dma_start(out=bias_t, in_=bias_d[:])
            i128b_t = consts.tile([128, 128], f16)
            nc.sync.dma_start(out=i128b_t, in_=i128b_d[:])
            ones1_t = consts.tile([1, 128], f16)
            nc.sync.dma_start(out=ones1_t, in_=ones1_d[:])

            # ---- all batch loads dispatched up front (per g half) ----
            x_tiles = []
            for b in range(BPC):
                x_t = xp.tile([128, 2, HW], f32, tag="x")
                x_tiles.append(x_t)
                for u in range(2):
                    for g in range(2):
                        nc.sync.dma_start(
                            out=x_t[:, g, u * 2048 : (u + 1) * 2048],
                            in_=xs_d[b, 128 * g : 128 * (g + 1)].rearrange(
                                "c h w -> c (h w)"
                            )[:, u * 2048 : (u + 1) * 2048],
                        )

            # Dummy matmuls absorb const-load DMA waits on PE early.
            pd = pss.tile([1, 4], f32, tag="sm")
            for k, (lhs, rhs) in enumerate((
                (w1t_t[:, 0, 0:1], w1t_t[:, 0, 1:2]),
                (wd_t[:, 0, 0:1], wd_t[:, 0, 1:2]),
                (i128b_t[:, 0:1], i128b_t[:, 1:2]),
            )):
                nc.tensor.matmul(pd[:, k : k + 1], lhs, rhs, start=True, stop=True)
            junk0 = work.tile([128, 1], f32, tag="junk")
            nc.scalar.activation(out=junk0, in_=bias_t, func=ACT.Copy)


            for b in range(BPC):
                x_t = x_tiles[b]

                # ---- ACT cast f32->f16 + ECA channel sums over hw ----
                # Split into 2048-wide pieces: long ACT ops otherwise block
                # the (high-priority) tail ops of the previous batch.
                scr = scrp.tile([128, 2, HW], f16, tag="scr")
                yb8 = work.tile([128, 2, 2], f32, tag="yb8")
                for u in range(2):
                    for g in range(2):
                        sl = slice(u * 2048, (u + 1) * 2048)
                        nc.scalar.activation(
                            out=scr[:, g, sl],
                            in_=x_t[:, g, sl],
                            func=ACT.Copy,
                            accum_out=yb8[:, g, u : u + 1],
                        )
                yb = work.tile([128, 2], f32, tag="yb")
                nc.vector.tensor_add(yb, yb8[:, :, 0], yb8[:, :, 1])

                # ---- DVE g-folds (split in halves, same reason) ----
                m1 = m1p.tile([128, HW], f16, tag="m1")
                for u in range(2):
                    sl = slice(u * 2048, (u + 1) * 2048)
                    nc.vector.tensor_max(m1[:, sl], scr[:, 0, sl], scr[:, 1, sl])

                # ---- transpose-reduce: 32-group partition reduction ----
                # R[32a+i, v] = red_j in[32a+j, 32v+i]   (pixel hw = 32v+i)
                # Split by v-halves for scheduling granularity.
                Rm = work.tile([128, 128], f16, tag="Rm")
                Ra = work.tile([128, 128], f32, tag="Ra")
                for u in range(2):
                    nc.vector.tensor_reduce(
                        out=Rm[:, u * 64 : (u + 1) * 64],
                        in_=m1[:, u * 2048 : (u + 1) * 2048].rearrange(
                            "p (v j) -> p v j", j=32
                        ),
                        axis=AX.X, op=ALU.max, apply_transpose=True,
                    )
                for u in range(2):
                    sl = slice(u * 2048, (u + 1) * 2048)
                    nc.vector.tensor_add(scr[:, 0, sl], scr[:, 0, sl], scr[:, 1, sl])
                for u in range(2):
                    nc.vector.tensor_reduce(
                        out=Ra[:, u * 64 : (u + 1) * 64],
                        in_=scr[:, 0, u * 2048 : (u + 1) * 2048].rearrange(
                            "p (v j) -> p v j", j=32
                        ),
                        axis=AX.X, op=ALU.add, apply_transpose=True,
                    )

                # Per-batch double-buffered conv maps: removes the
                # cross-batch WAR edge (minis(b+1) vs conv(b)).  Pads
                # (cols 0:2, 34:36) zeroed on ACT each batch.
                mp_map = mapp.tile([128, 36], f16, tag="mp")
                ap_map = mapp.tile([128, 36], f16, tag="ap")
                for mp in (mp_map, ap_map):
                    nc.scalar.memzero(mp[:, 0:2])
                    nc.scalar.memzero(mp[:, 34:36])

                # ---- finish reduction over the 4 a-groups; build maps ----
                # map[32q+i, 2+h2] = red_a R[32a+i, 4h2+q]
                # The whole per-batch tail (small map ops -> conv -> S' ->
                # multiply -> store) runs at high priority so the scheduler
                # drains batch b's latency chain before batch b+1's bulk
                # fold/reduce work.
                tail_ctx = tc.high_priority()
                tail_ctx.__enter__()
                for path, (R, top) in enumerate(((Rm, ALU.max), (Ra, ALU.add))):
                    G = work.tile([32, 4, 128], f16, tag=f"G{path}")
                    for a in range(4):
                        nc.scalar.activation(
                            out=G[:, a, :], in_=R[32 * a : 32 * a + 32, :],
                            func=ACT.Copy,
                        )
                    G2 = work.tile([32, 2, 128], f16, tag=f"G2{path}")
                    nc.vector.tensor_tensor(
                        out=G2.rearrange("p t v -> p (t v)"),
                        in0=G[:, 0:2, :].rearrange("p t v -> p (t v)"),
                        in1=G[:, 2:4, :].rearrange("p t v -> p (t v)"),
                        op=top,
                    )
                    # Final fold writes map stripes directly (output at
                    # partition base 32q): map[32q+i, 2+h2] = red over t of
                    # G2[i, t, 4h2+q].
                    mp = mp_map if path == 0 else ap_map
                    for q in range(4):
                        nc.vector.tensor_tensor(
                            out=mp[32 * q : 32 * q + 32, 2:34],
                            in0=G2[:, 0, q : 125 + q : 4],
                            in1=G2[:, 1, q : 125 + q : 4],
                            op=top,
                        )

                # ---- spatial conv (10 accumulated matmuls) + sigmoid ----
                psp = pss.tile([128, 32], f32, tag="sm")
                for ch in range(2):
                    mm = mp_map if ch == 0 else ap_map
                    for di in range(5):
                        nc.tensor.matmul(
                            psp,
                            wd_t[:, ch * 5 + di, :],
                            mm[:, di : di + 32],
                            start=(ch == 0 and di == 0),
                            stop=(ch == 1 and di == 4),
                        )
                sp16 = work.tile([128, 32], f16, tag="sp16")
                nc.scalar.activation(
                    out=sp16, in_=psp, func=ACT.Sigmoid, bias=bias_t[:, 0:1]
                )

                # ---- sp row: transpose + flatten DMA -> rrow [1, HW] ----
                pspT = pss.tile([32, 128], f16, tag="sm")
                nc.tensor.transpose(out=pspT, in_=sp16, identity=i128b_t)
                fr = work.tile([32, 128], f16, tag="fr")
                nc.scalar.activation(out=fr, in_=pspT, func=ACT.Identity, bias=1.0)
                rrow = rrp.tile([1, HW], f16, tag="rrow")
                nc.sync.dma_start(out=rrow, in_=fr)

                # ---- ECA conv1d + sigmoid -> chw1 = 1 + sigmoid(...) ----
                ybh = work.tile([128, 2], f16, tag="ybh")
                nc.scalar.activation(out=ybh, in_=yb, func=ACT.Copy)
                pchw = pss.tile([128, 2], f32, tag="sm")
                for hp in range(2):
                    for kh in range(2):
                        nc.tensor.matmul(
                            pchw[:, hp : hp + 1],
                            w1t_t[:, kh, hp * 128 : (hp + 1) * 128],
                            ybh[:, kh : kh + 1],
                            start=(kh == 0),
                            stop=(kh == 1),
                        )
                chw1 = work.tile([128, 2], f32, tag="chw")
                nc.scalar.activation(out=chw1, in_=pchw, func=ACT.Sigmoid)

                # ---- S' broadcast (PE, per 1024) + DVE stt multiply ----
                # stt: out = (S_psum + chw1_g) * x, in place over x.  All
                # multiplies on DVE (the dominant stream); no SBUF S' copies.
                tail_ctx.__exit__(None, None, None)
                for g in range(2):
                    for cc in range(2):
                        if g == 1 and cc == 1 and b < BPC - 1:
                            # Pool chunk: PSUM -> SBUF via ACT (bias adds
                            # 1 + chw), gpsimd multiply, store
                            xsl = x_t[:, g, cc * 2048 : (cc + 1) * 2048]
                            Sb = ssb.tile([128, 2048], f32, tag="Sb")
                            for half in range(2):
                                ps = psb.tile([128, 1024], f32, tag="bc")
                                for k in range(2):
                                    col = cc * 2048 + half * 1024 + k * 512
                                    nc.tensor.matmul(
                                        ps[:, k * 512 : (k + 1) * 512],
                                        ones1_t,
                                        rrow[:, col : col + 512],
                                        start=True, stop=True,
                                    )
                                nc.scalar.activation(
                                    out=Sb[:, half * 1024 : (half + 1) * 1024],
                                    in_=ps,
                                    func=ACT.Identity,
                                    bias=chw1[:, g : g + 1],
                                )
                            nc.gpsimd.tensor_mul(xsl, xsl, Sb)
                            nc.sync.dma_start(
                                out=out_d[b, 128 * g : 128 * (g + 1)]
                                .rearrange("c h w -> c (h w)")
                                [:, cc * 2048 : (cc + 1) * 2048],
                                in_=xsl,
                            )
                            continue
                        for half in range(2):
                            ps = psb.tile([128, 1024], f32, tag="bc")
                            for k in range(2):
                                col = cc * 2048 + half * 1024 + k * 512
                                nc.tensor.matmul(
                                    ps[:, k * 512 : (k + 1) * 512],
                                    ones1_t,
                                    rrow[:, col : col + 512],
                                    start=True, stop=True,
                                )
                            xh = x_t[:, g, cc * 2048 + half * 1024 :
                                     cc * 2048 + (half + 1) * 1024]
                            nc.vector.scalar_tensor_tensor(
                                out=xh,
                                in0=ps,
                                scalar=chw1[:, g : g + 1],
                                in1=xh,
                                op0=ALU.add,
                                op1=ALU.mult,
                            )
                        nc.sync.dma_start(
                            out=out_d[b, 128 * g : 128 * (g + 1)]
                            .rearrange("c h w -> c (h w)")
                            [:, cc * 2048 : (cc + 1) * 2048],
                            in_=x_t[:, g, cc * 2048 : (cc + 1) * 2048],
                        )

    _split_excess_waits(nc, mybir)
    return nc


def _split_excess_waits(nc, mybir):
    """Walrus limits sync-wait commands per instruction.  Move excess waits
    onto an inserted same-engine NoOp immediately before the instruction."""
    SKIP = (mybir.InstNoOp, mybir.InstAllEngineBarrier)
    for fn in nc.m.functions:
        for blk in fn.blocks:
            new = []
            for inst in blk.instructions:
                si = inst.sync_info
                if si is not None and si.on_wait and not isinstance(inst, SKIP):
                    waits = list(si.on_wait)
                    if len(waits) > 1:
                        moved, keep = waits[:-1], waits[-1:]
                        for k, w in enumerate(moved):
                            nop = mybir.InstNoOp(
                                name=f"{inst.name}-wsplit{k}",
                                engine=inst.engine,
                                sync_info=mybir.SyncInfo(on_wait=[w], on_update=[]),
                                bass_nofuse=True,
                            )
                            new.append(nop)
                        si.on_wait = keep
                new.append(inst)
            blk.instructions[:] = new


def kernel(x, conv1d_w, conv2d_w, conv2d_b):
    x = np.ascontiguousarray(np.asarray(x, np.float32))
    w1t, wd, bias = _build_host_consts(conv1d_w, conv2d_w, conv2d_b)

    from concourse.bass_utils import run_bass_kernel_spmd

    nc = _build_nc()
    shards = x.reshape(N_CORES, BPC, C, H, W)
    in_maps = [
        {"xs": np.ascontiguousarray(shards[i]), "w1t": w1t, "wd": wd, "bias": bias}
        for i in range(N_CORES)
    ]
    res = run_bass_kernel_spmd(nc, in_maps, core_ids=list(range(N_CORES)))
    out = np.concatenate([r["out"] for r in res.results], axis=0)
    return out.reshape(B, C, H, W)
